# revision 1
# baseline (speedup 1.0000x reference)
"""Trainium2 Bass kernel for a DGL-style InteractionNetwork (GNN message passing).

Strategy (edge-parallel, zero collectives):
  * Host sorts edges by receiver. Each of the 8 cores owns a contiguous
    12,500-node range and exactly the edges whose receiver falls in it, so the
    segment-sum is core-local and no all-reduce is needed.
  * Device, per core:
      Phase A: m_s[v] = [node_feat[v], 1] @ [We1_send; be1]  for all nodes
               (bf16 table in DRAM, gathered per-edge later).
      Phase B: per 128-edge slice: indirect-DMA gather m_s[senders];
               edge-feature term ef @ We1_edge on PE; receiver term via a
               128-node block one-hot matmul (receivers are block-local after
               sorting); relu; then aggregate the *hidden* layer into
               per-block PSUM with the same one-hot (hagg).
      Phase C: node MLP in fp32r. We2 is folded into the node MLP:
               agg @ Wn1_a = hagg @ (We2 @ Wn1_a) + deg x (be2 @ Wn1_a).
  * Host transposes/concats per-core outputs back to [100000, 64] f32.
"""

import numpy as np
import ml_dtypes

BF = ml_dtypes.bfloat16

N_NODES = 100000
N_EDGES = 1000000
D = 64
HID = 128
CORES = 8
NLOC = N_NODES // CORES            # 12500
BLK = 128
NBLK = (NLOC + BLK - 1) // BLK     # 98
NLOC_PAD = NBLK * BLK              # 12544
NFULL_SL = (N_NODES + 127) // 128  # 782
NFULL_PAD = NFULL_SL * 128         # 100096
PAD_RB = 200.0                     # one-hot-miss sentinel for pad edges

_prog_cache = {}


def _build(S):
    import concourse.mybir as mybir
    import concourse.tile as tile
    import concourse.bass as bass
    from concourse import bacc

    bf16 = mybir.dt.bfloat16
    f32 = mybir.dt.float32
    f32r = mybir.dt.float32r
    i32 = mybir.dt.int32
    Relu = mybir.ActivationFunctionType.Relu
    Ident = mybir.ActivationFunctionType.Identity

    T = NBLK * S

    nc = bacc.Bacc("TRN2", target_bir_lowering=False, debug=False,
                   num_devices=CORES)

    ef_t = nc.dram_tensor("ef_t", [64, T * 128], bf16, kind="ExternalInput")
    sidx_t = nc.dram_tensor("sidx_t", [128, T], i32, kind="ExternalInput")
    rb_t = nc.dram_tensor("rb_t", [128, T], f32, kind="ExternalInput")
    nf1_t = nc.dram_tensor("nf1_t", [65, NFULL_PAD], bf16, kind="ExternalInput")
    nfloc_b = nc.dram_tensor("nfloc_b", [64, NLOC_PAD], bf16, kind="ExternalInput")
    nfloc_f = nc.dram_tensor("nfloc_f", [64, NLOC_PAD], f32r, kind="ExternalInput")
    deg_d = nc.dram_tensor("deg", [1, NLOC_PAD], bf16, kind="ExternalInput")
    we1e_d = nc.dram_tensor("we1e", [64, HID], bf16, kind="ExternalInput")
    we1r_d = nc.dram_tensor("we1r", [64, HID], bf16, kind="ExternalInput")
    we1s_d = nc.dram_tensor("we1s1", [65, HID], bf16, kind="ExternalInput")
    wh1_d = nc.dram_tensor("wh1", [HID, HID], f32r, kind="ExternalInput")
    wn1n_d = nc.dram_tensor("wn1n", [64, HID], f32r, kind="ExternalInput")
    c1_d = nc.dram_tensor("c1", [1, HID], bf16, kind="ExternalInput")
    bn1_d = nc.dram_tensor("bn1c", [HID, 1], f32, kind="ExternalInput")
    wn2_d = nc.dram_tensor("wn2", [HID, D], f32r, kind="ExternalInput")
    bn2_d = nc.dram_tensor("bn2c", [D, 1], f32, kind="ExternalInput")
    iota_d = nc.dram_tensor("iota", [128, 128], bf16, kind="ExternalInput")
    id_d = nc.dram_tensor("ident", [128, 128], bf16, kind="ExternalInput")
    out_d = nc.dram_tensor("out_t", [64, NLOC_PAD], f32, kind="ExternalOutput")

    with tile.TileContext(nc) as tc:
        with tc.tile_pool(name="const", bufs=1) as cp, \
             tc.tile_pool(name="dram", bufs=1, space="DRAM") as dp, \
             tc.tile_pool(name="work", bufs=12) as wp, \
             tc.tile_pool(name="big", bufs=3) as bp:

            def cload(d, shape, dtype, tag):
                t = cp.tile(shape, dtype, tag=tag)
                nc.sync.dma_start(t[:], d[:])
                return t

            we1e = cload(we1e_d, [64, HID], bf16, "we1e")
            we1r = cload(we1r_d, [64, HID], bf16, "we1r")
            we1s = cload(we1s_d, [65, HID], bf16, "we1s")
            wh1 = cload(wh1_d, [HID, HID], f32r, "wh1")
            wn1n = cload(wn1n_d, [64, HID], f32r, "wn1n")
            c1 = cload(c1_d, [1, HID], bf16, "c1")
            bn1 = cload(bn1_d, [HID, 1], f32, "bn1")
            wn2 = cload(wn2_d, [HID, D], f32r, "wn2")
            bn2 = cload(bn2_d, [D, 1], f32, "bn2")
            iota = cload(iota_d, [128, 128], bf16, "iota")
            ident = cload(id_d, [128, 128], bf16, "ident")
            nfloc_sb = cload(nfloc_b, [64, NLOC_PAD], bf16, "nflocb")

            hagg = cp.tile([HID, NLOC_PAD], f32r, tag="hagg")
            m_s = dp.tile([NFULL_PAD, HID], bf16, tag="ms")

            # ---- Phase A: sender-hidden table m_s = [nf,1] @ [We1_s; be1] ----
            with tc.tile_pool(name="psA", bufs=4, space="PSUM") as psA:
                for j0 in range(0, NFULL_SL, 4):
                    take = min(4, NFULL_SL - j0)
                    nfa = wp.tile([65, 512], bf16, tag="nfa")
                    nc.sync.dma_start(nfa[:, :take * 128],
                                      nf1_t[:, j0 * 128:(j0 + take) * 128])
                    for i in range(take):
                        pm = psA.tile([128, HID], f32, tag="pm")
                        nc.tensor.matmul(out=pm[:],
                                         lhsT=nfa[:, i * 128:(i + 1) * 128],
                                         rhs=we1s[:], start=True, stop=True)
                        msb = wp.tile([128, HID], bf16, tag="msb")
                        nc.scalar.copy(out=msb[:], in_=pm[:])
                        r0 = (j0 + i) * 128
                        nc.sync.dma_start(m_s[r0:r0 + 128, :], msb[:])

            # ---- Phase B: edge MLP + hidden aggregation ----
            with tc.tile_pool(name="psBh", bufs=2, space="PSUM") as psBh, \
                 tc.tile_pool(name="psBa", bufs=3, space="PSUM") as psBa, \
                 tc.tile_pool(name="psBt", bufs=2, space="PSUM") as psBt, \
                 tc.tile_pool(name="psBp", bufs=1, space="PSUM") as psBp:
                for b in range(NBLK):
                    pP = psBp.tile([BLK, HID], f32, tag="pP")
                    nc.tensor.matmul(out=pP[:],
                                     lhsT=nfloc_sb[:, b * BLK:(b + 1) * BLK],
                                     rhs=we1r[:], start=True, stop=True)
                    Pb = wp.tile([BLK, HID], bf16, tag="Pb")
                    nc.vector.tensor_copy(out=Pb[:], in_=pP[:])

                    rbc = wp.tile([128, S], f32, tag="rbc")
                    nc.sync.dma_start(rbc[:], rb_t[:, b * S:(b + 1) * S])
                    sic = wp.tile([128, S], i32, tag="sic")
                    nc.sync.dma_start(sic[:], sidx_t[:, b * S:(b + 1) * S])
                    efc = bp.tile([64, S * 128], bf16, tag="efc")
                    nc.sync.dma_start(efc[:],
                                      ef_t[:, b * S * 128:(b + 1) * S * 128])

                    ph_agg = psBh.tile([HID, BLK], f32, tag="phagg")
                    for s in range(S):
                        ms_t = wp.tile([128, HID], bf16, tag="ms_t")
                        nc.gpsimd.indirect_dma_start(
                            out=ms_t[:], out_offset=None, in_=m_s[:, :],
                            in_offset=bass.IndirectOffsetOnAxis(
                                ap=sic[:, s:s + 1], axis=0))
                        oh_en = wp.tile([128, 128], bf16, tag="oh_en")
                        nc.vector.tensor_scalar(
                            out=oh_en[:], in0=iota[:], scalar1=rbc[:, s:s + 1],
                            scalar2=None, op0=mybir.AluOpType.is_equal)
                        pt = psBt.tile([128, 128], bf16, tag="pt")
                        nc.tensor.transpose(out=pt[:], in_=oh_en[:],
                                            identity=ident[:])
                        oh_ne = wp.tile([128, 128], bf16, tag="oh_ne")
                        nc.vector.tensor_copy(out=oh_ne[:], in_=pt[:])

                        ph = psBa.tile([128, HID], f32, tag="ph")
                        nc.tensor.matmul(out=ph[:],
                                         lhsT=efc[:, s * 128:(s + 1) * 128],
                                         rhs=we1e[:], start=True, stop=False)
                        nc.tensor.matmul(out=ph[:], lhsT=oh_ne[:], rhs=Pb[:],
                                         start=False, stop=True)
                        th = wp.tile([128, HID], f32, tag="th")
                        nc.vector.tensor_tensor(out=th[:], in0=ph[:],
                                                in1=ms_t[:],
                                                op=mybir.AluOpType.add)
                        hid = wp.tile([128, HID], bf16, tag="hid")
                        nc.scalar.activation(out=hid[:], in_=th[:], func=Relu)
                        nc.tensor.matmul(out=ph_agg[:], lhsT=hid[:],
                                         rhs=oh_en[:], start=(s == 0),
                                         stop=(s == S - 1))
                    nc.vector.tensor_copy(out=hagg[:, b * BLK:(b + 1) * BLK],
                                          in_=ph_agg[:])

            # ---- Phase C: node MLP (fp32r) ----
            with tc.tile_pool(name="psC", bufs=2, space="PSUM") as psC, \
                 tc.tile_pool(name="psCo", bufs=2, space="PSUM") as psCo:
                CH = 512
                for n0 in range(0, NLOC_PAD, CH):
                    cn = min(CH, NLOC_PAD - n0)
                    nfc = wp.tile([64, CH], f32r, tag="nfc")
                    nc.sync.dma_start(nfc[:, :cn], nfloc_f[:, n0:n0 + cn])
                    dgc = wp.tile([1, CH], bf16, tag="dgc")
                    nc.sync.dma_start(dgc[:, :cn], deg_d[:, n0:n0 + cn])
                    p1 = psC.tile([HID, CH], f32, tag="p1")
                    nc.tensor.matmul(out=p1[:, :cn], lhsT=wh1[:],
                                     rhs=hagg[:, n0:n0 + cn],
                                     start=True, stop=False)
                    nc.tensor.matmul(out=p1[:, :cn], lhsT=wn1n[:],
                                     rhs=nfc[:, :cn],
                                     start=False, stop=False)
                    nc.tensor.matmul(out=p1[:, :cn], lhsT=c1[:],
                                     rhs=dgc[:, :cn], start=False, stop=True)
                    nh = wp.tile([HID, CH], f32r, tag="nh")
                    nc.scalar.activation(out=nh[:, :cn], in_=p1[:, :cn],
                                         func=Relu, bias=bn1[:, 0:1])
                    po = psCo.tile([D, CH], f32, tag="po")
                    nc.tensor.matmul(out=po[:, :cn], lhsT=wn2[:],
                                     rhs=nh[:, :cn], start=True, stop=True)
                    oc = wp.tile([D, CH], f32, tag="oc")
                    nc.scalar.activation(out=oc[:, :cn], in_=po[:, :cn],
                                         func=Ident, bias=bn2[:, 0:1])
                    nc.sync.dma_start(out_d[:, n0:n0 + cn], oc[:, :cn])

    nc.compile()
    return nc


def _host_prep(inputs):
    nf = np.ascontiguousarray(np.asarray(inputs["node_feat"], dtype=np.float32))
    ef = np.ascontiguousarray(np.asarray(inputs["edge_feat"], dtype=np.float32))
    snd = np.asarray(inputs["senders"]).astype(np.int64)
    rcv = np.asarray(inputs["receivers"]).astype(np.int64)
    We1 = np.asarray(inputs["We1"], dtype=np.float32)
    be1 = np.asarray(inputs["be1"], dtype=np.float32)
    We2 = np.asarray(inputs["We2"], dtype=np.float32)
    be2 = np.asarray(inputs["be2"], dtype=np.float32)
    Wn1 = np.asarray(inputs["Wn1"], dtype=np.float32)
    bn1 = np.asarray(inputs["bn1"], dtype=np.float32)
    Wn2 = np.asarray(inputs["Wn2"], dtype=np.float32)
    bn2 = np.asarray(inputs["bn2"], dtype=np.float32)

    perm = np.argsort(rcv, kind="stable")
    rs = rcv[perm]
    ss = snd[perm].astype(np.int32)
    ef_s = ef[perm]

    bounds = np.searchsorted(rs, np.arange(CORES + 1) * NLOC)

    S = 1
    core_meta = []
    for c in range(CORES):
        lo, hi = int(bounds[c]), int(bounds[c + 1])
        r_loc = (rs[lo:hi] - c * NLOC).astype(np.int64)
        blk = r_loc >> 7
        cnts = np.bincount(blk, minlength=NBLK)
        if cnts.size:
            S = max(S, int(np.ceil(cnts.max() / 128.0)))
        core_meta.append((lo, hi, r_loc, blk, cnts))

    T = NBLK * S
    EPAD = T * 128

    bf = BF
    nf1_t = np.zeros((65, NFULL_PAD), dtype=bf)
    nf1_t[:64, :N_NODES] = nf.T.astype(bf)
    nf1_t[64, :] = np.ones((NFULL_PAD,), dtype=bf)
    we1e = np.ascontiguousarray(We1[0:64]).astype(bf)
    we1r = np.ascontiguousarray(We1[64:128]).astype(bf)
    we1s1 = np.concatenate([We1[128:192], be1[None, :]], axis=0).astype(bf)
    wh1 = np.ascontiguousarray(We2 @ Wn1[:64]).astype(np.float32)
    wn1n = np.ascontiguousarray(Wn1[64:128]).astype(np.float32)
    c1 = np.ascontiguousarray((be2 @ Wn1[:64])[None, :]).astype(bf)
    bn1c = np.ascontiguousarray(bn1[:, None]).astype(np.float32)
    wn2 = np.ascontiguousarray(Wn2).astype(np.float32)
    bn2c = np.ascontiguousarray(bn2[:, None]).astype(np.float32)
    iota = np.ascontiguousarray(
        np.broadcast_to(np.arange(128, dtype=np.float32)[None, :], (128, 128))
    ).astype(bf)
    ident = np.eye(128, dtype=bf)
    deg_full = np.bincount(rcv, minlength=N_NODES).astype(np.float32)

    in_maps = []
    for c in range(CORES):
        lo, hi, r_loc, blk, cnts = core_meta[c]
        ne = hi - lo
        starts = np.zeros(NBLK, dtype=np.int64)
        starts[1:] = np.cumsum(cnts)[:-1]
        within = np.arange(ne, dtype=np.int64) - starts[blk]
        slot = blk * (S * 128) + within

        ef_pad = np.zeros((EPAD, 64), dtype=np.float32)
        ef_pad[slot] = ef_s[lo:hi]
        ef_tc = np.ascontiguousarray(ef_pad.T).astype(bf)

        sidx = np.zeros((EPAD,), dtype=np.int32)
        sidx[slot] = ss[lo:hi]
        sidx_t = np.ascontiguousarray(sidx.reshape(T, 128).T)

        rb = np.full((EPAD,), PAD_RB, dtype=np.float32)
        rb[slot] = (r_loc - (blk << 7)).astype(np.float32)
        rb_t = np.ascontiguousarray(rb.reshape(T, 128).T)

        nfl = np.zeros((64, NLOC_PAD), dtype=np.float32)
        nfl[:, :NLOC] = nf[c * NLOC:(c + 1) * NLOC].T
        nfloc_f = np.ascontiguousarray(nfl)
        nfloc_b = nfloc_f.astype(bf)

        deg = np.zeros((1, NLOC_PAD), dtype=bf)
        deg[0, :NLOC] = deg_full[c * NLOC:(c + 1) * NLOC].astype(bf)

        in_maps.append({
            "ef_t": ef_tc, "sidx_t": sidx_t, "rb_t": rb_t,
            "nf1_t": nf1_t, "nfloc_b": nfloc_b, "nfloc_f": nfloc_f,
            "deg": deg, "we1e": we1e, "we1r": we1r, "we1s1": we1s1,
            "wh1": wh1, "wn1n": wn1n, "c1": c1, "bn1c": bn1c,
            "wn2": wn2, "bn2c": bn2c, "iota": iota, "ident": ident,
        })
    return S, in_maps


def _run(inputs, trace=False):
    from concourse.bass_utils import run_bass_kernel_spmd

    S, in_maps = _host_prep(inputs)
    if S not in _prog_cache:
        _prog_cache[S] = _build(S)
    nc = _prog_cache[S]
    res = run_bass_kernel_spmd(nc, in_maps, core_ids=list(range(CORES)),
                               trace=trace)
    out = np.empty((N_NODES, D), dtype=np.float32)
    for c in range(CORES):
        out[c * NLOC:(c + 1) * NLOC] = \
            np.asarray(res.results[c]["out_t"])[:, :NLOC].T
    return out, res


def kernel(**inputs):
    out, _ = _run(inputs, trace=False)
    return out



# revision 4
# speedup vs baseline: 1.1232x; 1.1232x over previous
"""Trainium2 Bass kernel for a DGL-style InteractionNetwork (GNN message passing).

Strategy v2 (edge-parallel, zero collectives, zero device-side gather):
  * Host permutes nodes into 784 balanced 128-node blocks (LPT bin-packing on
    degree) so every block owns <= S*128 edges with S minimal (10). 98 blocks
    per core; the per-core segment-sum is core-local (no all-reduce).
  * Host gathers sender/receiver node features into edge-slot order, so the
    device sees three dense bf16 streams and never does an indirect DMA:
      stream1[:, e] = [ef_e | nf[send_e]]        (128 rows)
      stream2[:, e] = [nf[recv_e] | 1]           (65 rows; 0 for pad slots)
  * Device, per 128-edge slice: two PSUM-accumulated matmuls (K=128, K=65)
    give the edge-MLP hidden pre-activation; relu (alternating ACT/DVE);
    one-hot segment-sum of the hidden layer into per-block PSUM on the PE.
  * Node MLP with We2 folded in:  agg @ Wn1_a = hagg @ (We2 @ Wn1_a)
    + deg * (be2 @ Wn1_a);  then relu, Wn2, bn2.
  * Host scatters per-core outputs back through the node permutation.
"""

import numpy as np
import ml_dtypes

BF = ml_dtypes.bfloat16

N_NODES = 100000
N_EDGES = 1000000
D = 64
HID = 128
CORES = 8
BLK = 128
NBLK = 98                          # blocks per core
NBLK_ALL = NBLK * CORES            # 784
NLOC_PAD = NBLK * BLK              # 12544 node slots per core
PAD_RB = 200.0                     # one-hot-miss sentinel for pad edges

_prog_cache = {}


def _build(S):
    import concourse.mybir as mybir
    import concourse.tile as tile
    from concourse import bacc

    bf16 = mybir.dt.bfloat16
    f32 = mybir.dt.float32
    f32r = mybir.dt.float32r
    Relu = mybir.ActivationFunctionType.Relu
    Ident = mybir.ActivationFunctionType.Identity
    EQ = mybir.AluOpType.is_equal
    MAX = mybir.AluOpType.max

    T = NBLK * S                   # total 128-edge slices per core

    nc = bacc.Bacc("TRN2", target_bir_lowering=False, debug=False,
                   num_devices=CORES)

    s1_d = nc.dram_tensor("s1", [128, T * 128], bf16, kind="ExternalInput")
    s2_d = nc.dram_tensor("s2", [65, T * 128], bf16, kind="ExternalInput")
    rb_d = nc.dram_tensor("rb", [128, T], f32, kind="ExternalInput")
    nfloc_d = nc.dram_tensor("nfloc", [64, NLOC_PAD], f32r, kind="ExternalInput")
    deg_d = nc.dram_tensor("deg", [1, NLOC_PAD], bf16, kind="ExternalInput")
    wa_d = nc.dram_tensor("wa", [128, HID], bf16, kind="ExternalInput")
    wb_d = nc.dram_tensor("wb", [65, HID], bf16, kind="ExternalInput")
    wh1_d = nc.dram_tensor("wh1", [HID, HID], f32r, kind="ExternalInput")
    wn1n_d = nc.dram_tensor("wn1n", [64, HID], f32r, kind="ExternalInput")
    c1_d = nc.dram_tensor("c1", [1, HID], bf16, kind="ExternalInput")
    bn1_d = nc.dram_tensor("bn1c", [HID, 1], f32, kind="ExternalInput")
    wn2_d = nc.dram_tensor("wn2", [HID, D], f32r, kind="ExternalInput")
    bn2_d = nc.dram_tensor("bn2c", [D, 1], f32, kind="ExternalInput")
    iota_d = nc.dram_tensor("iota", [128, 128], bf16, kind="ExternalInput")
    out_d = nc.dram_tensor("out_t", [64, NLOC_PAD], f32, kind="ExternalOutput")

    SB = 2                          # blocks per stream-DMA chunk
    CW = SB * S * 128               # stream columns per chunk

    with tile.TileContext(nc) as tc:
        with tc.tile_pool(name="const", bufs=1) as cp, \
             tc.tile_pool(name="s1p", bufs=3) as s1p, \
             tc.tile_pool(name="s2p", bufs=3) as s2p, \
             tc.tile_pool(name="work", bufs=4) as wp:

            def cload(d, shape, dtype, tag):
                t = cp.tile(shape, dtype, tag=tag)
                nc.sync.dma_start(t[:], d[:])
                return t

            wa = cload(wa_d, [128, HID], bf16, "wa")
            wb = cload(wb_d, [65, HID], bf16, "wb")
            iota = cload(iota_d, [128, 128], bf16, "iota")
            rball = cload(rb_d, [128, T], f32, "rball")
            wh1 = cload(wh1_d, [HID, HID], f32r, "wh1")
            wn1n = cload(wn1n_d, [64, HID], f32r, "wn1n")
            c1 = cload(c1_d, [1, HID], bf16, "c1")
            bn1 = cload(bn1_d, [HID, 1], f32, "bn1")
            wn2 = cload(wn2_d, [HID, D], f32r, "wn2")
            bn2 = cload(bn2_d, [D, 1], f32, "bn2")
            nfloc = cload(nfloc_d, [64, NLOC_PAD], f32r, "nfloc")
            degall = cload(deg_d, [1, NLOC_PAD], bf16, "degall")

            hagg = cp.tile([HID, NLOC_PAD], f32r, tag="hagg")

            # ---- Phase B: edge MLP layer 1 + hidden segment-sum ----
            # The hidden-aggregation matmul for slice t is issued DELAY slices
            # late so the PE never waits on the relu latency.
            DELAY = 2
            with tc.tile_pool(name="psB", bufs=4, space="PSUM") as psB, \
                 tc.tile_pool(name="psA", bufs=2, space="PSUM") as psA, \
                 tc.tile_pool(name="ohp", bufs=DELAY + 3) as ohp, \
                 tc.tile_pool(name="hidp", bufs=DELAY + 3) as hidp:
                pend = []
                agg_tile = [None]

                def issue_agg(hid_, oh_, b_, s_):
                    if s_ == 0:
                        agg_tile[0] = psA.tile([HID, BLK], f32, tag="ph_agg",
                                               name="ph_agg")
                    pa = agg_tile[0]
                    nc.tensor.matmul(out=pa[:], lhsT=hid_[:], rhs=oh_[:],
                                     start=(s_ == 0), stop=(s_ == S - 1))
                    if s_ == S - 1:
                        if b_ % 2 == 0:
                            nc.vector.tensor_copy(
                                out=hagg[:, b_ * BLK:(b_ + 1) * BLK],
                                in_=pa[:])
                        else:
                            nc.scalar.copy(
                                out=hagg[:, b_ * BLK:(b_ + 1) * BLK],
                                in_=pa[:])

                for b in range(NBLK):
                    bi = b % SB
                    if bi == 0:
                        s1c = s1p.tile([128, CW], bf16, tag="s1c")
                        nc.sync.dma_start(s1c[:], s1_d[:, b * S * 128:
                                                       b * S * 128 + CW])
                        s2c = s2p.tile([65, CW], bf16, tag="s2c")
                        nc.sync.dma_start(s2c[:], s2_d[:, b * S * 128:
                                                       b * S * 128 + CW])
                    for s in range(S):
                        col = (bi * S + s) * 128
                        g = b * S + s
                        oh = ohp.tile([128, 128], bf16, tag="oh")
                        nc.gpsimd.tensor_scalar(
                            out=oh[:], in0=iota[:],
                            scalar1=rball[:, g:g + 1], scalar2=None,
                            op0=EQ)
                        ph = psB.tile([128, HID], f32, tag="ph")
                        nc.tensor.matmul(out=ph[:],
                                         lhsT=s1c[:, col:col + 128],
                                         rhs=wa[:], start=True, stop=False)
                        nc.tensor.matmul(out=ph[:],
                                         lhsT=s2c[:, col:col + 128],
                                         rhs=wb[:], start=False, stop=True)
                        hid = hidp.tile([128, HID], bf16, tag="hid")
                        if s % 2 == 0:
                            nc.scalar.activation(out=hid[:], in_=ph[:],
                                                 func=Relu)
                        else:
                            nc.vector.tensor_scalar(
                                out=hid[:], in0=ph[:], scalar1=0.0,
                                scalar2=None, op0=MAX)
                        pend.append((hid, oh, b, s))
                        if len(pend) > DELAY:
                            issue_agg(*pend.pop(0))
                while pend:
                    issue_agg(*pend.pop(0))

            # ---- Phase C: node MLP (fp32r) ----
            with tc.tile_pool(name="psC", bufs=2, space="PSUM") as psC, \
                 tc.tile_pool(name="psCo", bufs=2, space="PSUM") as psCo:
                CH = 512
                for n0 in range(0, NLOC_PAD, CH):
                    cn = min(CH, NLOC_PAD - n0)
                    p1 = psC.tile([HID, CH], f32, tag="p1")
                    nc.tensor.matmul(out=p1[:, :cn], lhsT=wh1[:],
                                     rhs=hagg[:, n0:n0 + cn],
                                     start=True, stop=False)
                    nc.tensor.matmul(out=p1[:, :cn], lhsT=wn1n[:],
                                     rhs=nfloc[:, n0:n0 + cn],
                                     start=False, stop=False)
                    nc.tensor.matmul(out=p1[:, :cn], lhsT=c1[:],
                                     rhs=degall[:, n0:n0 + cn],
                                     start=False, stop=True)
                    nh = wp.tile([HID, CH], f32r, tag="nh")
                    nc.scalar.activation(out=nh[:, :cn], in_=p1[:, :cn],
                                         func=Relu, bias=bn1[:, 0:1])
                    po = psCo.tile([D, CH], f32, tag="po")
                    nc.tensor.matmul(out=po[:, :cn], lhsT=wn2[:],
                                     rhs=nh[:, :cn], start=True, stop=True)
                    oc = wp.tile([D, CH], f32, tag="oc")
                    nc.scalar.activation(out=oc[:, :cn], in_=po[:, :cn],
                                         func=Ident, bias=bn2[:, 0:1])
                    nc.sync.dma_start(out_d[:, n0:n0 + cn], oc[:, :cn])

    nc.compile()
    return nc


def _balance_blocks(deg):
    """LPT bin-packing: nodes -> 784 blocks of <=128 nodes, balancing the
    per-block edge (receiver) totals. Returns (block_of_node, slot_of_node,
    max_block_load)."""
    import heapq
    order = np.argsort(-deg, kind="stable")
    heap = [(0, 0, b) for b in range(NBLK_ALL)]
    heapq.heapify(heap)
    block_of = np.empty(N_NODES, dtype=np.int32)
    slot_of = np.empty(N_NODES, dtype=np.int32)
    deg_l = deg.tolist()
    maxload = 0
    for n in order.tolist():
        load, cnt, b = heapq.heappop(heap)
        block_of[n] = b
        slot_of[n] = cnt
        load += deg_l[n]
        cnt += 1
        if load > maxload:
            maxload = load
        if cnt < BLK:
            heapq.heappush(heap, (load, cnt, b))
    return block_of, slot_of, maxload


def _host_prep(inputs):
    nf = np.ascontiguousarray(np.asarray(inputs["node_feat"], dtype=np.float32))
    ef = np.ascontiguousarray(np.asarray(inputs["edge_feat"], dtype=np.float32))
    snd = np.asarray(inputs["senders"]).astype(np.int64)
    rcv = np.asarray(inputs["receivers"]).astype(np.int64)
    We1 = np.asarray(inputs["We1"], dtype=np.float32)
    be1 = np.asarray(inputs["be1"], dtype=np.float32)
    We2 = np.asarray(inputs["We2"], dtype=np.float32)
    be2 = np.asarray(inputs["be2"], dtype=np.float32)
    Wn1 = np.asarray(inputs["Wn1"], dtype=np.float32)
    bn1 = np.asarray(inputs["bn1"], dtype=np.float32)
    Wn2 = np.asarray(inputs["Wn2"], dtype=np.float32)
    bn2 = np.asarray(inputs["bn2"], dtype=np.float32)

    deg_full = np.bincount(rcv, minlength=N_NODES).astype(np.int64)
    block_of, slot_of, maxload = _balance_blocks(deg_full)
    S = max(1, int(np.ceil(maxload / 128.0)))
    T = NBLK * S
    EPAD = T * 128

    core_of = (block_of // NBLK).astype(np.int32)       # node -> core
    blk_loc = (block_of % NBLK).astype(np.int64)        # node -> block in core
    pos_of = blk_loc * BLK + slot_of                    # node -> slot in core

    # per-edge routing (by receiver)
    e_core = core_of[rcv]
    e_blk = blk_loc[rcv]
    e_rb = slot_of[rcv].astype(np.float32)

    bf = BF
    wa = np.concatenate([We1[0:64], We1[128:192]], axis=0).astype(bf)
    wb = np.concatenate([We1[64:128], be1[None, :]], axis=0).astype(bf)
    wh1 = np.ascontiguousarray(We2 @ Wn1[:64]).astype(np.float32)
    wn1n = np.ascontiguousarray(Wn1[64:128]).astype(np.float32)
    c1 = np.ascontiguousarray((be2 @ Wn1[:64])[None, :]).astype(bf)
    bn1c = np.ascontiguousarray(bn1[:, None]).astype(np.float32)
    wn2 = np.ascontiguousarray(Wn2).astype(np.float32)
    bn2c = np.ascontiguousarray(bn2[:, None]).astype(np.float32)
    iota = np.ascontiguousarray(
        np.broadcast_to(np.arange(128, dtype=np.float32)[None, :], (128, 128))
    ).astype(bf)

    in_maps = []
    for c in range(CORES):
        sel = np.nonzero(e_core == c)[0]
        blk = e_blk[sel]
        order = np.argsort(blk, kind="stable")
        sel = sel[order]
        blk = blk[order]
        cnts = np.bincount(blk, minlength=NBLK)
        starts = np.zeros(NBLK, dtype=np.int64)
        starts[1:] = np.cumsum(cnts)[:-1]
        within = np.arange(sel.size, dtype=np.int64) - starts[blk]
        col = blk * (S * 128) + within

        s1 = np.zeros((128, EPAD), dtype=bf)
        s1[0:64, col] = ef[sel].T
        s1[64:128, col] = nf[snd[sel]].T
        s2 = np.zeros((65, EPAD), dtype=bf)
        s2[0:64, col] = nf[rcv[sel]].T
        s2[64, col] = 1.0

        rbv = np.full((EPAD,), PAD_RB, dtype=np.float32)
        rbv[col] = e_rb[sel]
        rb_t = np.ascontiguousarray(rbv.reshape(T, 128).T)

        mine = np.nonzero(core_of == c)[0]
        nfloc = np.zeros((64, NLOC_PAD), dtype=np.float32)
        nfloc[:, pos_of[mine]] = nf[mine].T
        degl = np.zeros((1, NLOC_PAD), dtype=bf)
        degl[0, pos_of[mine]] = deg_full[mine].astype(bf)

        in_maps.append({
            "s1": s1, "s2": s2, "rb": rb_t, "nfloc": nfloc, "deg": degl,
            "wa": wa, "wb": wb, "wh1": wh1, "wn1n": wn1n, "c1": c1,
            "bn1c": bn1c, "wn2": wn2, "bn2c": bn2c, "iota": iota,
        })

    gpos = core_of.astype(np.int64) * NLOC_PAD + pos_of
    return S, in_maps, gpos


def _run(inputs, trace=False):
    from concourse.bass_utils import run_bass_kernel_spmd

    S, in_maps, gpos = _host_prep(inputs)
    if S not in _prog_cache:
        _prog_cache[S] = _build(S)
    nc = _prog_cache[S]
    res = run_bass_kernel_spmd(nc, in_maps, core_ids=list(range(CORES)),
                               trace=trace)
    big = np.concatenate(
        [np.asarray(res.results[c]["out_t"]) for c in range(CORES)], axis=1)
    out = np.ascontiguousarray(big[:, gpos].T.astype(np.float32))
    return out, res


def kernel(**inputs):
    out, _ = _run(inputs, trace=False)
    return out


# revision 5
# speedup vs baseline: 8.1255x; 7.2343x over previous
"""Trainium2 Bass kernel for a DGL-style InteractionNetwork (GNN message passing).

Strategy v2 (edge-parallel, zero collectives, zero device-side gather):
  * Host permutes nodes into 784 balanced 128-node blocks (LPT bin-packing on
    degree) so every block owns <= S*128 edges with S minimal (10). 98 blocks
    per core; the per-core segment-sum is core-local (no all-reduce).
  * Host gathers sender/receiver node features into edge-slot order, so the
    device sees three dense bf16 streams and never does an indirect DMA:
      stream1[:, e] = [ef_e | nf[send_e]]        (128 rows)
      stream2[:, e] = [nf[recv_e] | 1]           (65 rows; 0 for pad slots)
  * Device, per 128-edge slice: two PSUM-accumulated matmuls (K=128, K=65)
    give the edge-MLP hidden pre-activation; relu (alternating ACT/DVE);
    one-hot segment-sum of the hidden layer into per-block PSUM on the PE.
  * Node MLP with We2 folded in:  agg @ Wn1_a = hagg @ (We2 @ Wn1_a)
    + deg * (be2 @ Wn1_a);  then relu, Wn2, bn2.
  * Host scatters per-core outputs back through the node permutation.
"""

import numpy as np
import ml_dtypes

BF = ml_dtypes.bfloat16

N_NODES = 100000
N_EDGES = 1000000
D = 64
HID = 128
CORES = 8
BLK = 128
NBLK = 98                          # blocks per core
NBLK_ALL = NBLK * CORES            # 784
NLOC_PAD = NBLK * BLK              # 12544 node slots per core
PAD_RB = 200.0                     # one-hot-miss sentinel for pad edges

_prog_cache = {}


def _build(S):
    import concourse.mybir as mybir
    import concourse.tile as tile
    from concourse import bacc

    bf16 = mybir.dt.bfloat16
    f32 = mybir.dt.float32
    f32r = mybir.dt.float32r
    Relu = mybir.ActivationFunctionType.Relu
    Ident = mybir.ActivationFunctionType.Identity
    EQ = mybir.AluOpType.is_equal
    MAX = mybir.AluOpType.max

    T = NBLK * S                   # total 128-edge slices per core

    nc = bacc.Bacc("TRN2", target_bir_lowering=False, debug=False,
                   num_devices=CORES)

    s1_d = nc.dram_tensor("s1", [128, T * 128], bf16, kind="ExternalInput")
    s2_d = nc.dram_tensor("s2", [65, T * 128], bf16, kind="ExternalInput")
    rb_d = nc.dram_tensor("rb", [128, T], f32, kind="ExternalInput")
    nfloc_d = nc.dram_tensor("nfloc", [64, NLOC_PAD], f32r, kind="ExternalInput")
    deg_d = nc.dram_tensor("deg", [1, NLOC_PAD], bf16, kind="ExternalInput")
    wa_d = nc.dram_tensor("wa", [128, HID], bf16, kind="ExternalInput")
    wb_d = nc.dram_tensor("wb", [65, HID], bf16, kind="ExternalInput")
    wh1_d = nc.dram_tensor("wh1", [HID, HID], f32r, kind="ExternalInput")
    wn1n_d = nc.dram_tensor("wn1n", [64, HID], f32r, kind="ExternalInput")
    c1_d = nc.dram_tensor("c1", [1, HID], bf16, kind="ExternalInput")
    bn1_d = nc.dram_tensor("bn1c", [HID, 1], f32, kind="ExternalInput")
    wn2_d = nc.dram_tensor("wn2", [HID, D], f32r, kind="ExternalInput")
    bn2_d = nc.dram_tensor("bn2c", [D, 1], f32, kind="ExternalInput")
    iota_d = nc.dram_tensor("iota", [128, 128], bf16, kind="ExternalInput")
    out_d = nc.dram_tensor("out_t", [64, NLOC_PAD], f32, kind="ExternalOutput")

    SB = 2 if S % 2 == 0 else 4     # blocks per stream-DMA chunk
    CW = SB * S * 128               # stream columns per chunk
    GR = 4                          # slices per wide-relu group (PSUM bank)

    with tile.TileContext(nc) as tc:
        with tc.tile_pool(name="const", bufs=1) as cp, \
             tc.tile_pool(name="s1p", bufs=3) as s1p, \
             tc.tile_pool(name="s2p", bufs=3) as s2p, \
             tc.tile_pool(name="work", bufs=4) as wp:

            def cload(d, shape, dtype, tag):
                t = cp.tile(shape, dtype, tag=tag)
                nc.sync.dma_start(t[:], d[:])
                return t

            wa = cload(wa_d, [128, HID], bf16, "wa")
            wb = cload(wb_d, [65, HID], bf16, "wb")
            iota = cload(iota_d, [128, 128], bf16, "iota")
            rball = cload(rb_d, [128, T], f32, "rball")
            wh1 = cload(wh1_d, [HID, HID], f32r, "wh1")
            wn1n = cload(wn1n_d, [64, HID], f32r, "wn1n")
            c1 = cload(c1_d, [1, HID], bf16, "c1")
            bn1 = cload(bn1_d, [HID, 1], f32, "bn1")
            wn2 = cload(wn2_d, [HID, D], f32r, "wn2")
            bn2 = cload(bn2_d, [D, 1], f32, "bn2")
            nfloc = cload(nfloc_d, [64, NLOC_PAD], f32r, "nfloc")
            degall = cload(deg_d, [1, NLOC_PAD], bf16, "degall")

            hagg = cp.tile([HID, NLOC_PAD], f32r, tag="hagg")

            # ---- Phase B: edge MLP layer 1 + hidden segment-sum ----
            # GR slices share one PSUM bank so the relu runs as a single wide
            # ACT op; the hidden-aggregation matmuls for a group are issued
            # one group late so the PE never waits on the relu latency.
            T_ALL = NBLK * S
            assert T_ALL % GR == 0 and (SB * S) % GR == 0
            with tc.tile_pool(name="psB", bufs=3, space="PSUM") as psB, \
                 tc.tile_pool(name="psA", bufs=2, space="PSUM") as psA, \
                 tc.tile_pool(name="ohp", bufs=2 * GR + 2) as ohp, \
                 tc.tile_pool(name="hidp", bufs=3) as hidp:
                pend = []
                agg_tile = [None]

                def issue_agg(hidw_, r_, oh_, b_, s_):
                    if s_ == 0:
                        agg_tile[0] = psA.tile([HID, BLK], f32, tag="ph_agg",
                                               name="ph_agg")
                    pa = agg_tile[0]
                    nc.tensor.matmul(out=pa[:],
                                     lhsT=hidw_[:, r_ * 128:(r_ + 1) * 128],
                                     rhs=oh_[:], start=(s_ == 0),
                                     stop=(s_ == S - 1))
                    if s_ == S - 1:
                        nc.vector.tensor_copy(
                            out=hagg[:, b_ * BLK:(b_ + 1) * BLK],
                            in_=pa[:])

                s1c = s2c = None
                for g in range(T_ALL // GR):
                    phw = psB.tile([128, GR * HID], f32, tag="phw")
                    hidw = hidp.tile([128, GR * HID], bf16, tag="hidw")
                    grp = []
                    for r in range(GR):
                        t = g * GR + r
                        b, s = divmod(t, S)
                        if t % (SB * S) == 0:
                            s1c = s1p.tile([128, CW], bf16, tag="s1c")
                            nc.sync.dma_start(s1c[:], s1_d[:, t * 128:
                                                           t * 128 + CW])
                            s2c = s2p.tile([65, CW], bf16, tag="s2c")
                            nc.sync.dma_start(s2c[:], s2_d[:, t * 128:
                                                           t * 128 + CW])
                        col = (t % (SB * S)) * 128
                        oh = ohp.tile([128, 128], bf16, tag="oh")
                        nc.vector.tensor_scalar(
                            out=oh[:], in0=iota[:],
                            scalar1=rball[:, t:t + 1], scalar2=None,
                            op0=EQ)
                        ph = phw[:, r * HID:(r + 1) * HID]
                        nc.tensor.matmul(out=ph,
                                         lhsT=s1c[:, col:col + 128],
                                         rhs=wa[:], start=True, stop=False)
                        nc.tensor.matmul(out=ph,
                                         lhsT=s2c[:, col:col + 128],
                                         rhs=wb[:], start=False, stop=True)
                        grp.append((hidw, r, oh, b, s))
                    nc.scalar.activation(out=hidw[:], in_=phw[:], func=Relu)
                    pend.extend(grp)
                    if len(pend) > GR:
                        for _ in range(GR):
                            issue_agg(*pend.pop(0))
                while pend:
                    issue_agg(*pend.pop(0))

            # ---- Phase C: node MLP (fp32r) ----
            with tc.tile_pool(name="psC", bufs=2, space="PSUM") as psC, \
                 tc.tile_pool(name="psCo", bufs=2, space="PSUM") as psCo:
                CH = 512
                for n0 in range(0, NLOC_PAD, CH):
                    cn = min(CH, NLOC_PAD - n0)
                    p1 = psC.tile([HID, CH], f32, tag="p1")
                    nc.tensor.matmul(out=p1[:, :cn], lhsT=wh1[:],
                                     rhs=hagg[:, n0:n0 + cn],
                                     start=True, stop=False)
                    nc.tensor.matmul(out=p1[:, :cn], lhsT=wn1n[:],
                                     rhs=nfloc[:, n0:n0 + cn],
                                     start=False, stop=False)
                    nc.tensor.matmul(out=p1[:, :cn], lhsT=c1[:],
                                     rhs=degall[:, n0:n0 + cn],
                                     start=False, stop=True)
                    nh = wp.tile([HID, CH], f32r, tag="nh")
                    nc.scalar.activation(out=nh[:, :cn], in_=p1[:, :cn],
                                         func=Relu, bias=bn1[:, 0:1])
                    po = psCo.tile([D, CH], f32, tag="po")
                    nc.tensor.matmul(out=po[:, :cn], lhsT=wn2[:],
                                     rhs=nh[:, :cn], start=True, stop=True)
                    oc = wp.tile([D, CH], f32, tag="oc")
                    nc.scalar.activation(out=oc[:, :cn], in_=po[:, :cn],
                                         func=Ident, bias=bn2[:, 0:1])
                    nc.sync.dma_start(out_d[:, n0:n0 + cn], oc[:, :cn])

    nc.compile()
    return nc


def _balance_blocks(deg):
    """LPT bin-packing: nodes -> 784 blocks of <=128 nodes, balancing the
    per-block edge (receiver) totals. Returns (block_of_node, slot_of_node,
    max_block_load)."""
    import heapq
    order = np.argsort(-deg, kind="stable")
    heap = [(0, 0, b) for b in range(NBLK_ALL)]
    heapq.heapify(heap)
    block_of = np.empty(N_NODES, dtype=np.int32)
    slot_of = np.empty(N_NODES, dtype=np.int32)
    deg_l = deg.tolist()
    maxload = 0
    for n in order.tolist():
        load, cnt, b = heapq.heappop(heap)
        block_of[n] = b
        slot_of[n] = cnt
        load += deg_l[n]
        cnt += 1
        if load > maxload:
            maxload = load
        if cnt < BLK:
            heapq.heappush(heap, (load, cnt, b))
    return block_of, slot_of, maxload


def _host_prep(inputs):
    nf = np.ascontiguousarray(np.asarray(inputs["node_feat"], dtype=np.float32))
    ef = np.ascontiguousarray(np.asarray(inputs["edge_feat"], dtype=np.float32))
    snd = np.asarray(inputs["senders"]).astype(np.int64)
    rcv = np.asarray(inputs["receivers"]).astype(np.int64)
    We1 = np.asarray(inputs["We1"], dtype=np.float32)
    be1 = np.asarray(inputs["be1"], dtype=np.float32)
    We2 = np.asarray(inputs["We2"], dtype=np.float32)
    be2 = np.asarray(inputs["be2"], dtype=np.float32)
    Wn1 = np.asarray(inputs["Wn1"], dtype=np.float32)
    bn1 = np.asarray(inputs["bn1"], dtype=np.float32)
    Wn2 = np.asarray(inputs["Wn2"], dtype=np.float32)
    bn2 = np.asarray(inputs["bn2"], dtype=np.float32)

    deg_full = np.bincount(rcv, minlength=N_NODES).astype(np.int64)
    block_of, slot_of, maxload = _balance_blocks(deg_full)
    S = max(1, int(np.ceil(maxload / 128.0)))
    T = NBLK * S
    EPAD = T * 128

    core_of = (block_of // NBLK).astype(np.int32)       # node -> core
    blk_loc = (block_of % NBLK).astype(np.int64)        # node -> block in core
    pos_of = blk_loc * BLK + slot_of                    # node -> slot in core

    # per-edge routing (by receiver)
    e_core = core_of[rcv]
    e_blk = blk_loc[rcv]
    e_rb = slot_of[rcv].astype(np.float32)

    bf = BF
    wa = np.concatenate([We1[0:64], We1[128:192]], axis=0).astype(bf)
    wb = np.concatenate([We1[64:128], be1[None, :]], axis=0).astype(bf)
    wh1 = np.ascontiguousarray(We2 @ Wn1[:64]).astype(np.float32)
    wn1n = np.ascontiguousarray(Wn1[64:128]).astype(np.float32)
    c1 = np.ascontiguousarray((be2 @ Wn1[:64])[None, :]).astype(bf)
    bn1c = np.ascontiguousarray(bn1[:, None]).astype(np.float32)
    wn2 = np.ascontiguousarray(Wn2).astype(np.float32)
    bn2c = np.ascontiguousarray(bn2[:, None]).astype(np.float32)
    iota = np.ascontiguousarray(
        np.broadcast_to(np.arange(128, dtype=np.float32)[None, :], (128, 128))
    ).astype(bf)

    in_maps = []
    for c in range(CORES):
        sel = np.nonzero(e_core == c)[0]
        blk = e_blk[sel]
        order = np.argsort(blk, kind="stable")
        sel = sel[order]
        blk = blk[order]
        cnts = np.bincount(blk, minlength=NBLK)
        starts = np.zeros(NBLK, dtype=np.int64)
        starts[1:] = np.cumsum(cnts)[:-1]
        within = np.arange(sel.size, dtype=np.int64) - starts[blk]
        col = blk * (S * 128) + within

        s1 = np.zeros((128, EPAD), dtype=bf)
        s1[0:64, col] = ef[sel].T
        s1[64:128, col] = nf[snd[sel]].T
        s2 = np.zeros((65, EPAD), dtype=bf)
        s2[0:64, col] = nf[rcv[sel]].T
        s2[64, col] = 1.0

        rbv = np.full((EPAD,), PAD_RB, dtype=np.float32)
        rbv[col] = e_rb[sel]
        rb_t = np.ascontiguousarray(rbv.reshape(T, 128).T)

        mine = np.nonzero(core_of == c)[0]
        nfloc = np.zeros((64, NLOC_PAD), dtype=np.float32)
        nfloc[:, pos_of[mine]] = nf[mine].T
        degl = np.zeros((1, NLOC_PAD), dtype=bf)
        degl[0, pos_of[mine]] = deg_full[mine].astype(bf)

        in_maps.append({
            "s1": s1, "s2": s2, "rb": rb_t, "nfloc": nfloc, "deg": degl,
            "wa": wa, "wb": wb, "wh1": wh1, "wn1n": wn1n, "c1": c1,
            "bn1c": bn1c, "wn2": wn2, "bn2c": bn2c, "iota": iota,
        })

    gpos = core_of.astype(np.int64) * NLOC_PAD + pos_of
    return S, in_maps, gpos


def _run(inputs, trace=False):
    from concourse.bass_utils import run_bass_kernel_spmd

    S, in_maps, gpos = _host_prep(inputs)
    if S not in _prog_cache:
        _prog_cache[S] = _build(S)
    nc = _prog_cache[S]
    res = run_bass_kernel_spmd(nc, in_maps, core_ids=list(range(CORES)),
                               trace=trace)
    big = np.concatenate(
        [np.asarray(res.results[c]["out_t"]) for c in range(CORES)], axis=1)
    out = np.ascontiguousarray(big[:, gpos].T.astype(np.float32))
    return out, res


def kernel(**inputs):
    out, _ = _run(inputs, trace=False)
    return out


# revision 6
# speedup vs baseline: 8.5161x; 1.0481x over previous
"""Trainium2 Bass kernel for a DGL-style InteractionNetwork (GNN message passing).

Strategy v4 (edge-parallel, zero collectives, zero device-side gather):
  * Host permutes nodes into 1600 balanced 64-node blocks (LPT bin-packing on
    receiver degree) so every block owns <= S*128 edges with S minimal (5).
    200 blocks per core; the per-core segment-sum is core-local.
  * Host gathers sender/receiver node features into edge-slot order, so the
    device sees dense bf16 streams and never does an indirect DMA:
      stream1[:, e] = [ef_e | nf[send_e]]        (128 rows)
      stream2[:, e] = [nf[recv_e] | 1]           (65 rows; 0 for pad slots)
  * Device, per 128-edge slice: two PSUM-accumulated matmuls (K=128, K=65)
    give the edge-MLP hidden pre-activation; one-hot (DVE is_equal) segment
    sum of the hidden layer on the PE.  4 slices share a PSUM bank so the
    relu is a single wide ACT op; aggregation matmuls trail one group so the
    PE never waits on the relu.
  * 8 blocks form a 512-node window in one PSUM bank; when a window
    completes, its node-MLP chunk (bf16, We2 folded in) is issued inline,
    two groups later, so phase C fully overlaps phase B.
  * Host scatters per-core outputs back through the node permutation.
"""

import numpy as np
import ml_dtypes

BF = ml_dtypes.bfloat16

N_NODES = 100000
N_EDGES = 1000000
D = 64
HID = 128
CORES = 8
BLK = 64                           # nodes per one-hot block
NBLK = 200                         # blocks per core
NBLK_ALL = NBLK * CORES            # 1600
NLOC_PAD = NBLK * BLK              # 12800 node slots per core
WIN = 8                            # blocks per PSUM aggregation window
CH = WIN * BLK                     # 512 nodes per phase-C chunk
PAD_RB = 200.0                     # one-hot-miss sentinel for pad edges

_prog_cache = {}


def _build(S):
    import concourse.mybir as mybir
    import concourse.tile as tile
    from concourse import bacc

    bf16 = mybir.dt.bfloat16
    f32 = mybir.dt.float32
    Relu = mybir.ActivationFunctionType.Relu
    Ident = mybir.ActivationFunctionType.Identity
    EQ = mybir.AluOpType.is_equal

    T = NBLK * S                   # total 128-edge slices per core
    SB = 4                         # blocks per stream-DMA chunk
    CW = SB * S * 128              # stream columns per chunk
    GR = 4                         # slices per wide-relu group (PSUM bank)
    assert T % GR == 0 and (SB * S) % GR == 0 and NBLK % WIN == 0

    nc = bacc.Bacc("TRN2", target_bir_lowering=False, debug=False,
                   num_devices=CORES)

    s1_d = nc.dram_tensor("s1", [128, T * 128], bf16, kind="ExternalInput")
    s2_d = nc.dram_tensor("s2", [65, T * 128], bf16, kind="ExternalInput")
    rb_d = nc.dram_tensor("rb", [128, T], f32, kind="ExternalInput")
    nfloc_d = nc.dram_tensor("nfloc", [64, NLOC_PAD], bf16, kind="ExternalInput")
    deg_d = nc.dram_tensor("deg", [1, NLOC_PAD], bf16, kind="ExternalInput")
    wa_d = nc.dram_tensor("wa", [128, HID], bf16, kind="ExternalInput")
    wb_d = nc.dram_tensor("wb", [65, HID], bf16, kind="ExternalInput")
    wh1_d = nc.dram_tensor("wh1", [HID, HID], bf16, kind="ExternalInput")
    wn1n_d = nc.dram_tensor("wn1n", [64, HID], bf16, kind="ExternalInput")
    c1_d = nc.dram_tensor("c1", [1, HID], bf16, kind="ExternalInput")
    bn1_d = nc.dram_tensor("bn1c", [HID, 1], f32, kind="ExternalInput")
    wn2_d = nc.dram_tensor("wn2", [HID, D], bf16, kind="ExternalInput")
    bn2_d = nc.dram_tensor("bn2c", [D, 1], f32, kind="ExternalInput")
    iota_d = nc.dram_tensor("iota", [128, BLK], bf16, kind="ExternalInput")
    out_d = nc.dram_tensor("out_t", [64, NLOC_PAD], f32, kind="ExternalOutput")

    with tile.TileContext(nc) as tc:
        with tc.tile_pool(name="const", bufs=1) as cp, \
             tc.tile_pool(name="s1p", bufs=3) as s1p, \
             tc.tile_pool(name="s2p", bufs=3) as s2p, \
             tc.tile_pool(name="hw", bufs=2) as hwp, \
             tc.tile_pool(name="wrk", bufs=4) as wp:

            def cload(d, shape, dtype, tag):
                t = cp.tile(shape, dtype, tag=tag)
                nc.sync.dma_start(t[:], d[:])
                return t

            wa = cload(wa_d, [128, HID], bf16, "wa")
            wb = cload(wb_d, [65, HID], bf16, "wb")
            iota = cload(iota_d, [128, BLK], bf16, "iota")
            rball = cload(rb_d, [128, T], f32, "rball")
            wh1 = cload(wh1_d, [HID, HID], bf16, "wh1")
            wn1n = cload(wn1n_d, [64, HID], bf16, "wn1n")
            c1 = cload(c1_d, [1, HID], bf16, "c1")
            bn1 = cload(bn1_d, [HID, 1], f32, "bn1")
            wn2 = cload(wn2_d, [HID, D], bf16, "wn2")
            bn2 = cload(bn2_d, [D, 1], f32, "bn2")
            nfloc = cload(nfloc_d, [64, NLOC_PAD], bf16, "nfloc")
            degall = cload(deg_d, [1, NLOC_PAD], bf16, "degall")

            with tc.tile_pool(name="psB", bufs=3, space="PSUM") as psB, \
                 tc.tile_pool(name="psA", bufs=2, space="PSUM") as psA, \
                 tc.tile_pool(name="psC", bufs=2, space="PSUM") as psC, \
                 tc.tile_pool(name="psCo", bufs=1, space="PSUM") as psCo, \
                 tc.tile_pool(name="ohp", bufs=2 * GR + 2) as ohp, \
                 tc.tile_pool(name="hidp", bufs=3) as hidp:

                pend = []          # slices whose agg matmul is not yet issued
                pend_c = []        # completed windows awaiting phase C
                agg_tile = [None]

                def phase_c(w, haggw):
                    n0 = w * CH
                    p1 = psC.tile([HID, CH], f32, tag="p1", name="p1")
                    nc.tensor.matmul(out=p1[:], lhsT=wh1[:], rhs=haggw[:],
                                     start=True, stop=False)
                    nc.tensor.matmul(out=p1[:], lhsT=wn1n[:],
                                     rhs=nfloc[:, n0:n0 + CH],
                                     start=False, stop=False)
                    nc.tensor.matmul(out=p1[:], lhsT=c1[:],
                                     rhs=degall[:, n0:n0 + CH],
                                     start=False, stop=True)
                    nh = wp.tile([HID, CH], bf16, tag="nh", name="nh")
                    nc.scalar.activation(out=nh[:], in_=p1[:],
                                         func=Relu, bias=bn1[:, 0:1])
                    po = psCo.tile([D, CH], f32, tag="po", name="po")
                    nc.tensor.matmul(out=po[:], lhsT=wn2[:], rhs=nh[:],
                                     start=True, stop=True)
                    oc = wp.tile([D, CH], f32, tag="oc", name="oc")
                    nc.scalar.activation(out=oc[:], in_=po[:],
                                         func=Ident, bias=bn2[:, 0:1])
                    nc.sync.dma_start(out_d[:, n0:n0 + CH], oc[:])

                def issue_agg(hidw_, r_, oh_, b_, s_):
                    wb_ = b_ % WIN
                    if wb_ == 0 and s_ == 0:
                        agg_tile[0] = psA.tile([HID, WIN * BLK], f32,
                                               tag="paw", name="paw")
                    pa = agg_tile[0]
                    nc.tensor.matmul(out=pa[:, wb_ * BLK:(wb_ + 1) * BLK],
                                     lhsT=hidw_[:, r_ * 128:(r_ + 1) * 128],
                                     rhs=oh_[:], start=(s_ == 0),
                                     stop=(s_ == S - 1))
                    if wb_ == WIN - 1 and s_ == S - 1:
                        haggw = hwp.tile([HID, WIN * BLK], bf16, tag="hagw",
                                         name="hagw")
                        nc.vector.tensor_copy(out=haggw[:], in_=pa[:])
                        pend_c.append([b_ // WIN, haggw, 2])

                s1c = s2c = None
                for g in range(T // GR):
                    # phase-C chunks issue two groups after their window ends
                    for item in pend_c:
                        item[2] -= 1
                    while pend_c and pend_c[0][2] <= 0:
                        w_, hg_, _ = pend_c.pop(0)
                        phase_c(w_, hg_)
                    phw = psB.tile([128, GR * HID], f32, tag="phw")
                    hidw = hidp.tile([128, GR * HID], bf16, tag="hidw")
                    grp = []
                    for r in range(GR):
                        t = g * GR + r
                        b, s = divmod(t, S)
                        if t % (SB * S) == 0:
                            s1c = s1p.tile([128, CW], bf16, tag="s1c")
                            nc.sync.dma_start(s1c[:], s1_d[:, t * 128:
                                                           t * 128 + CW])
                            s2c = s2p.tile([65, CW], bf16, tag="s2c")
                            nc.sync.dma_start(s2c[:], s2_d[:, t * 128:
                                                           t * 128 + CW])
                        col = (t % (SB * S)) * 128
                        oh = ohp.tile([128, BLK], bf16, tag="oh")
                        nc.vector.tensor_scalar(
                            out=oh[:], in0=iota[:],
                            scalar1=rball[:, t:t + 1], scalar2=None,
                            op0=EQ)
                        ph = phw[:, r * HID:(r + 1) * HID]
                        nc.tensor.matmul(out=ph,
                                         lhsT=s1c[:, col:col + 128],
                                         rhs=wa[:], start=True, stop=False)
                        nc.tensor.matmul(out=ph,
                                         lhsT=s2c[:, col:col + 128],
                                         rhs=wb[:], start=False, stop=True)
                        grp.append((hidw, r, oh, b, s))
                    nc.scalar.activation(out=hidw[:], in_=phw[:], func=Relu)
                    pend.extend(grp)
                    if len(pend) > GR:
                        for _ in range(GR):
                            issue_agg(*pend.pop(0))
                while pend:
                    issue_agg(*pend.pop(0))
                while pend_c:
                    w_, hg_, _ = pend_c.pop(0)
                    phase_c(w_, hg_)

    nc.compile()
    return nc


def _balance_blocks(deg):
    """LPT bin-packing: nodes -> NBLK_ALL blocks of <=BLK nodes, balancing the
    per-block edge (receiver) totals."""
    import heapq
    order = np.argsort(-deg, kind="stable")
    heap = [(0, 0, b) for b in range(NBLK_ALL)]
    heapq.heapify(heap)
    block_of = np.empty(N_NODES, dtype=np.int32)
    slot_of = np.empty(N_NODES, dtype=np.int32)
    deg_l = deg.tolist()
    maxload = 0
    for n in order.tolist():
        load, cnt, b = heapq.heappop(heap)
        block_of[n] = b
        slot_of[n] = cnt
        load += deg_l[n]
        cnt += 1
        if load > maxload:
            maxload = load
        if cnt < BLK:
            heapq.heappush(heap, (load, cnt, b))
    return block_of, slot_of, maxload


def _host_prep(inputs):
    nf = np.ascontiguousarray(np.asarray(inputs["node_feat"], dtype=np.float32))
    ef = np.ascontiguousarray(np.asarray(inputs["edge_feat"], dtype=np.float32))
    snd = np.asarray(inputs["senders"]).astype(np.int64)
    rcv = np.asarray(inputs["receivers"]).astype(np.int64)
    We1 = np.asarray(inputs["We1"], dtype=np.float32)
    be1 = np.asarray(inputs["be1"], dtype=np.float32)
    We2 = np.asarray(inputs["We2"], dtype=np.float32)
    be2 = np.asarray(inputs["be2"], dtype=np.float32)
    Wn1 = np.asarray(inputs["Wn1"], dtype=np.float32)
    bn1 = np.asarray(inputs["bn1"], dtype=np.float32)
    Wn2 = np.asarray(inputs["Wn2"], dtype=np.float32)
    bn2 = np.asarray(inputs["bn2"], dtype=np.float32)

    deg_full = np.bincount(rcv, minlength=N_NODES).astype(np.int64)
    block_of, slot_of, maxload = _balance_blocks(deg_full)
    S = max(1, int(np.ceil(maxload / 128.0)))
    T = NBLK * S
    EPAD = T * 128

    core_of = (block_of // NBLK).astype(np.int32)       # node -> core
    blk_loc = (block_of % NBLK).astype(np.int64)        # node -> block in core
    pos_of = blk_loc * BLK + slot_of                    # node -> slot in core

    # per-edge routing (by receiver)
    e_core = core_of[rcv]
    e_blk = blk_loc[rcv]
    e_rb = slot_of[rcv].astype(np.float32)

    bf = BF
    wa = np.concatenate([We1[0:64], We1[128:192]], axis=0).astype(bf)
    wb = np.concatenate([We1[64:128], be1[None, :]], axis=0).astype(bf)
    wh1 = np.ascontiguousarray(We2 @ Wn1[:64]).astype(bf)
    wn1n = np.ascontiguousarray(Wn1[64:128]).astype(bf)
    c1 = np.ascontiguousarray((be2 @ Wn1[:64])[None, :]).astype(bf)
    bn1c = np.ascontiguousarray(bn1[:, None]).astype(np.float32)
    wn2 = np.ascontiguousarray(Wn2).astype(bf)
    bn2c = np.ascontiguousarray(bn2[:, None]).astype(np.float32)
    iota = np.ascontiguousarray(
        np.broadcast_to(np.arange(BLK, dtype=np.float32)[None, :], (128, BLK))
    ).astype(bf)

    in_maps = []
    for c in range(CORES):
        sel = np.nonzero(e_core == c)[0]
        blk = e_blk[sel]
        order = np.argsort(blk, kind="stable")
        sel = sel[order]
        blk = blk[order]
        cnts = np.bincount(blk, minlength=NBLK)
        starts = np.zeros(NBLK, dtype=np.int64)
        starts[1:] = np.cumsum(cnts)[:-1]
        within = np.arange(sel.size, dtype=np.int64) - starts[blk]
        col = blk * (S * 128) + within

        s1 = np.zeros((128, EPAD), dtype=bf)
        s1[0:64, col] = ef[sel].T
        s1[64:128, col] = nf[snd[sel]].T
        s2 = np.zeros((65, EPAD), dtype=bf)
        s2[0:64, col] = nf[rcv[sel]].T
        s2[64, col] = 1.0

        rbv = np.full((EPAD,), PAD_RB, dtype=np.float32)
        rbv[col] = e_rb[sel]
        rb_t = np.ascontiguousarray(rbv.reshape(T, 128).T)

        mine = np.nonzero(core_of == c)[0]
        nfloc = np.zeros((64, NLOC_PAD), dtype=bf)
        nfloc[:, pos_of[mine]] = nf[mine].T
        degl = np.zeros((1, NLOC_PAD), dtype=bf)
        degl[0, pos_of[mine]] = deg_full[mine].astype(bf)

        in_maps.append({
            "s1": s1, "s2": s2, "rb": rb_t, "nfloc": nfloc, "deg": degl,
            "wa": wa, "wb": wb, "wh1": wh1, "wn1n": wn1n, "c1": c1,
            "bn1c": bn1c, "wn2": wn2, "bn2c": bn2c, "iota": iota,
        })

    gpos = core_of.astype(np.int64) * NLOC_PAD + pos_of
    return S, in_maps, gpos


def _run(inputs, trace=False):
    from concourse.bass_utils import run_bass_kernel_spmd

    S, in_maps, gpos = _host_prep(inputs)
    if S not in _prog_cache:
        _prog_cache[S] = _build(S)
    nc = _prog_cache[S]
    res = run_bass_kernel_spmd(nc, in_maps, core_ids=list(range(CORES)),
                               trace=trace)
    big = np.concatenate(
        [np.asarray(res.results[c]["out_t"]) for c in range(CORES)], axis=1)
    out = np.ascontiguousarray(big[:, gpos].T.astype(np.float32))
    return out, res


def kernel(**inputs):
    out, _ = _run(inputs, trace=False)
    return out


# revision 8
# speedup vs baseline: 10.0186x; 1.1764x over previous
"""Trainium2 Bass kernel for a DGL-style InteractionNetwork (GNN message passing).

Strategy v4 (edge-parallel, zero collectives, zero device-side gather):
  * Host permutes nodes into 1600 balanced 64-node blocks (LPT bin-packing on
    receiver degree) so every block owns <= S*128 edges with S minimal (5).
    200 blocks per core; the per-core segment-sum is core-local.
  * Host gathers sender/receiver node features into edge-slot order, so the
    device sees dense bf16 streams and never does an indirect DMA:
      stream1[:, e] = [ef_e | nf[send_e]]        (128 rows)
      stream2[:, e] = [nf[recv_e] | 1]           (65 rows; 0 for pad slots)
  * Device, per 128-edge slice: two PSUM-accumulated matmuls (K=128, K=65)
    give the edge-MLP hidden pre-activation; one-hot (DVE is_equal) segment
    sum of the hidden layer on the PE.  4 slices share a PSUM bank so the
    relu is a single wide ACT op; aggregation matmuls trail one group so the
    PE never waits on the relu.
  * 8 blocks form a 512-node window in one PSUM bank; when a window
    completes, its node-MLP chunk (bf16, We2 folded in) is issued inline,
    two groups later, so phase C fully overlaps phase B.
  * Host scatters per-core outputs back through the node permutation.
"""

import numpy as np
import ml_dtypes

BF = ml_dtypes.bfloat16

N_NODES = 100000
N_EDGES = 1000000
D = 64
HID = 128
CORES = 8
BLK = 64                           # nodes per one-hot block
NBLK = 200                         # blocks per core
NBLK_ALL = NBLK * CORES            # 1600
NLOC_PAD = NBLK * BLK              # 12800 node slots per core
WIN = 8                            # blocks per PSUM aggregation window
CH = WIN * BLK                     # 512 nodes per phase-C chunk
PAD_RB = 200.0                     # one-hot-miss sentinel for pad edges

_prog_cache = {}


def _build(S):
    import concourse.mybir as mybir
    import concourse.tile as tile
    from concourse import bacc

    bf16 = mybir.dt.bfloat16
    f32 = mybir.dt.float32
    Relu = mybir.ActivationFunctionType.Relu
    Ident = mybir.ActivationFunctionType.Identity
    EQ = mybir.AluOpType.is_equal

    T = NBLK * S                   # total 128-edge slices per core
    SB = 4                         # blocks per stream-DMA chunk
    CW = SB * S * 128              # stream columns per chunk
    GR = 4                         # slices per wide-relu group (PSUM bank)
    assert T % GR == 0 and (SB * S) % GR == 0 and NBLK % WIN == 0

    nc = bacc.Bacc("TRN2", target_bir_lowering=False, debug=False,
                   num_devices=CORES)

    s1_d = nc.dram_tensor("s1", [128, T * 128], bf16, kind="ExternalInput")
    s2_d = nc.dram_tensor("s2", [65, T * 128], bf16, kind="ExternalInput")
    rb_d = nc.dram_tensor("rb", [128, T], f32, kind="ExternalInput")
    nfloc_d = nc.dram_tensor("nfloc", [64, NLOC_PAD], bf16, kind="ExternalInput")
    deg_d = nc.dram_tensor("deg", [1, NLOC_PAD], bf16, kind="ExternalInput")
    wa_d = nc.dram_tensor("wa", [128, HID], bf16, kind="ExternalInput")
    wb_d = nc.dram_tensor("wb", [65, HID], bf16, kind="ExternalInput")
    wh1_d = nc.dram_tensor("wh1", [HID, HID], bf16, kind="ExternalInput")
    wn1n_d = nc.dram_tensor("wn1n", [64, HID], bf16, kind="ExternalInput")
    c1_d = nc.dram_tensor("c1", [1, HID], bf16, kind="ExternalInput")
    bn1_d = nc.dram_tensor("bn1c", [HID, 1], f32, kind="ExternalInput")
    wn2_d = nc.dram_tensor("wn2", [HID, D], bf16, kind="ExternalInput")
    bn2_d = nc.dram_tensor("bn2c", [D, 1], f32, kind="ExternalInput")
    iota_d = nc.dram_tensor("iota", [128, BLK], bf16, kind="ExternalInput")
    out_d = nc.dram_tensor("out_t", [64, NLOC_PAD], f32, kind="ExternalOutput")

    with tile.TileContext(nc) as tc:
        with tc.tile_pool(name="const", bufs=1) as cp, \
             tc.tile_pool(name="s1p", bufs=4) as s1p, \
             tc.tile_pool(name="s2p", bufs=4) as s2p, \
             tc.tile_pool(name="hw", bufs=3) as hwp, \
             tc.tile_pool(name="wrk", bufs=4) as wp:

            def cload(d, shape, dtype, tag, eng=None):
                t = cp.tile(shape, dtype, tag=tag)
                (eng or nc.sync).dma_start(t[:], d[:])
                return t

            # hot constants first on the SP ring so the first stream chunks
            # are not queued behind the big phase-C loads (those go on the
            # ACT ring instead).
            wa = cload(wa_d, [128, HID], bf16, "wa")
            wb = cload(wb_d, [65, HID], bf16, "wb")
            iota = cload(iota_d, [128, BLK], bf16, "iota")
            rball = cload(rb_d, [128, T], f32, "rball")
            wh1 = cload(wh1_d, [HID, HID], bf16, "wh1", nc.scalar)
            wn1n = cload(wn1n_d, [64, HID], bf16, "wn1n", nc.scalar)
            c1 = cload(c1_d, [1, HID], bf16, "c1", nc.scalar)
            bn1 = cload(bn1_d, [HID, 1], f32, "bn1", nc.scalar)
            wn2 = cload(wn2_d, [HID, D], bf16, "wn2", nc.scalar)
            bn2 = cload(bn2_d, [D, 1], f32, "bn2", nc.scalar)
            nfloc = cload(nfloc_d, [64, NLOC_PAD], bf16, "nfloc", nc.scalar)
            degall = cload(deg_d, [1, NLOC_PAD], bf16, "degall", nc.scalar)

            with tc.tile_pool(name="psB", bufs=3, space="PSUM") as psB, \
                 tc.tile_pool(name="psA", bufs=2, space="PSUM") as psA, \
                 tc.tile_pool(name="psC", bufs=2, space="PSUM") as psC, \
                 tc.tile_pool(name="psCo", bufs=1, space="PSUM") as psCo, \
                 tc.tile_pool(name="ohp", bufs=2 * GR + 2) as ohp, \
                 tc.tile_pool(name="hidp", bufs=3) as hidp:

                pend = []          # slices whose agg matmul is not yet issued
                pend_c = []        # completed windows awaiting phase C
                agg_tile = [None]

                def phase_c(w, haggw):
                    n0 = w * CH
                    p1 = psC.tile([HID, CH], f32, tag="p1", name="p1")
                    nc.tensor.matmul(out=p1[:], lhsT=wh1[:], rhs=haggw[:],
                                     start=True, stop=False)
                    nc.tensor.matmul(out=p1[:], lhsT=wn1n[:],
                                     rhs=nfloc[:, n0:n0 + CH],
                                     start=False, stop=False)
                    nc.tensor.matmul(out=p1[:], lhsT=c1[:],
                                     rhs=degall[:, n0:n0 + CH],
                                     start=False, stop=True)
                    nh = wp.tile([HID, CH], bf16, tag="nh", name="nh")
                    nc.scalar.activation(out=nh[:], in_=p1[:],
                                         func=Relu, bias=bn1[:, 0:1])
                    po = psCo.tile([D, CH], f32, tag="po", name="po")
                    nc.tensor.matmul(out=po[:], lhsT=wn2[:], rhs=nh[:],
                                     start=True, stop=True)
                    oc = wp.tile([D, CH], f32, tag="oc", name="oc")
                    nc.scalar.activation(out=oc[:], in_=po[:],
                                         func=Ident, bias=bn2[:, 0:1])
                    nc.sync.dma_start(out_d[:, n0:n0 + CH], oc[:])

                def issue_agg(hidw_, r_, oh_, b_, s_):
                    wb_ = b_ % WIN
                    if wb_ == 0 and s_ == 0:
                        agg_tile[0] = psA.tile([HID, WIN * BLK], f32,
                                               tag="paw", name="paw")
                    pa = agg_tile[0]
                    nc.tensor.matmul(out=pa[:, wb_ * BLK:(wb_ + 1) * BLK],
                                     lhsT=hidw_[:, r_ * 128:(r_ + 1) * 128],
                                     rhs=oh_[:], start=(s_ == 0),
                                     stop=(s_ == S - 1))
                    if wb_ == WIN - 1 and s_ == S - 1:
                        haggw = hwp.tile([HID, WIN * BLK], bf16, tag="hagw",
                                         name="hagw")
                        nc.vector.tensor_copy(out=haggw[:], in_=pa[:])
                        pend_c.append([b_ // WIN, haggw, 3])

                s1c = s2c = None
                for g in range(T // GR):
                    # phase-C chunks issue two groups after their window ends
                    for item in pend_c:
                        item[2] -= 1
                    while pend_c and pend_c[0][2] <= 0:
                        w_, hg_, _ = pend_c.pop(0)
                        phase_c(w_, hg_)
                    phw = psB.tile([128, GR * HID], f32, tag="phw")
                    hidw = hidp.tile([128, GR * HID], bf16, tag="hidw")
                    grp = []
                    for r in range(GR):
                        t = g * GR + r
                        b, s = divmod(t, S)
                        if t % (SB * S) == 0:
                            s1c = s1p.tile([128, CW], bf16, tag="s1c")
                            nc.sync.dma_start(s1c[:], s1_d[:, t * 128:
                                                           t * 128 + CW])
                            s2c = s2p.tile([65, CW], bf16, tag="s2c")
                            nc.gpsimd.dma_start(s2c[:], s2_d[:, t * 128:
                                                             t * 128 + CW])
                        col = (t % (SB * S)) * 128
                        oh = ohp.tile([128, BLK], bf16, tag="oh")
                        nc.vector.tensor_scalar(
                            out=oh[:], in0=iota[:],
                            scalar1=rball[:, t:t + 1], scalar2=None,
                            op0=EQ)
                        ph = phw[:, r * HID:(r + 1) * HID]
                        nc.tensor.matmul(out=ph,
                                         lhsT=s1c[:, col:col + 128],
                                         rhs=wa[:], start=True, stop=False)
                        nc.tensor.matmul(out=ph,
                                         lhsT=s2c[:, col:col + 128],
                                         rhs=wb[:], start=False, stop=True)
                        grp.append((hidw, r, oh, b, s))
                    nc.scalar.activation(out=hidw[:], in_=phw[:], func=Relu)
                    pend.extend(grp)
                    if len(pend) > GR:
                        for _ in range(GR):
                            issue_agg(*pend.pop(0))
                while pend:
                    issue_agg(*pend.pop(0))
                while pend_c:
                    w_, hg_, _ = pend_c.pop(0)
                    phase_c(w_, hg_)

    nc.compile()
    return nc


def _balance_blocks(deg):
    """LPT bin-packing: nodes -> NBLK_ALL blocks of <=BLK nodes, balancing the
    per-block edge (receiver) totals."""
    import heapq
    order = np.argsort(-deg, kind="stable")
    heap = [(0, 0, b) for b in range(NBLK_ALL)]
    heapq.heapify(heap)
    block_of = np.empty(N_NODES, dtype=np.int32)
    slot_of = np.empty(N_NODES, dtype=np.int32)
    deg_l = deg.tolist()
    maxload = 0
    for n in order.tolist():
        load, cnt, b = heapq.heappop(heap)
        block_of[n] = b
        slot_of[n] = cnt
        load += deg_l[n]
        cnt += 1
        if load > maxload:
            maxload = load
        if cnt < BLK:
            heapq.heappush(heap, (load, cnt, b))
    return block_of, slot_of, maxload


def _host_prep(inputs):
    nf = np.ascontiguousarray(np.asarray(inputs["node_feat"], dtype=np.float32))
    ef = np.ascontiguousarray(np.asarray(inputs["edge_feat"], dtype=np.float32))
    snd = np.asarray(inputs["senders"]).astype(np.int64)
    rcv = np.asarray(inputs["receivers"]).astype(np.int64)
    We1 = np.asarray(inputs["We1"], dtype=np.float32)
    be1 = np.asarray(inputs["be1"], dtype=np.float32)
    We2 = np.asarray(inputs["We2"], dtype=np.float32)
    be2 = np.asarray(inputs["be2"], dtype=np.float32)
    Wn1 = np.asarray(inputs["Wn1"], dtype=np.float32)
    bn1 = np.asarray(inputs["bn1"], dtype=np.float32)
    Wn2 = np.asarray(inputs["Wn2"], dtype=np.float32)
    bn2 = np.asarray(inputs["bn2"], dtype=np.float32)

    deg_full = np.bincount(rcv, minlength=N_NODES).astype(np.int64)
    block_of, slot_of, maxload = _balance_blocks(deg_full)
    S = max(1, int(np.ceil(maxload / 128.0)))
    T = NBLK * S
    EPAD = T * 128

    core_of = (block_of // NBLK).astype(np.int32)       # node -> core
    blk_loc = (block_of % NBLK).astype(np.int64)        # node -> block in core
    pos_of = blk_loc * BLK + slot_of                    # node -> slot in core

    # per-edge routing (by receiver)
    e_core = core_of[rcv]
    e_blk = blk_loc[rcv]
    e_rb = slot_of[rcv].astype(np.float32)

    bf = BF
    wa = np.concatenate([We1[0:64], We1[128:192]], axis=0).astype(bf)
    wb = np.concatenate([We1[64:128], be1[None, :]], axis=0).astype(bf)
    wh1 = np.ascontiguousarray(We2 @ Wn1[:64]).astype(bf)
    wn1n = np.ascontiguousarray(Wn1[64:128]).astype(bf)
    c1 = np.ascontiguousarray((be2 @ Wn1[:64])[None, :]).astype(bf)
    bn1c = np.ascontiguousarray(bn1[:, None]).astype(np.float32)
    wn2 = np.ascontiguousarray(Wn2).astype(bf)
    bn2c = np.ascontiguousarray(bn2[:, None]).astype(np.float32)
    iota = np.ascontiguousarray(
        np.broadcast_to(np.arange(BLK, dtype=np.float32)[None, :], (128, BLK))
    ).astype(bf)

    in_maps = []
    for c in range(CORES):
        sel = np.nonzero(e_core == c)[0]
        blk = e_blk[sel]
        order = np.argsort(blk, kind="stable")
        sel = sel[order]
        blk = blk[order]
        cnts = np.bincount(blk, minlength=NBLK)
        starts = np.zeros(NBLK, dtype=np.int64)
        starts[1:] = np.cumsum(cnts)[:-1]
        within = np.arange(sel.size, dtype=np.int64) - starts[blk]
        col = blk * (S * 128) + within

        s1 = np.zeros((128, EPAD), dtype=bf)
        s1[0:64, col] = ef[sel].T
        s1[64:128, col] = nf[snd[sel]].T
        s2 = np.zeros((65, EPAD), dtype=bf)
        s2[0:64, col] = nf[rcv[sel]].T
        s2[64, col] = 1.0

        rbv = np.full((EPAD,), PAD_RB, dtype=np.float32)
        rbv[col] = e_rb[sel]
        rb_t = np.ascontiguousarray(rbv.reshape(T, 128).T)

        mine = np.nonzero(core_of == c)[0]
        nfloc = np.zeros((64, NLOC_PAD), dtype=bf)
        nfloc[:, pos_of[mine]] = nf[mine].T
        degl = np.zeros((1, NLOC_PAD), dtype=bf)
        degl[0, pos_of[mine]] = deg_full[mine].astype(bf)

        in_maps.append({
            "s1": s1, "s2": s2, "rb": rb_t, "nfloc": nfloc, "deg": degl,
            "wa": wa, "wb": wb, "wh1": wh1, "wn1n": wn1n, "c1": c1,
            "bn1c": bn1c, "wn2": wn2, "bn2c": bn2c, "iota": iota,
        })

    gpos = core_of.astype(np.int64) * NLOC_PAD + pos_of
    return S, in_maps, gpos


def _run(inputs, trace=False):
    from concourse.bass_utils import run_bass_kernel_spmd

    S, in_maps, gpos = _host_prep(inputs)
    if S not in _prog_cache:
        _prog_cache[S] = _build(S)
    nc = _prog_cache[S]
    res = run_bass_kernel_spmd(nc, in_maps, core_ids=list(range(CORES)),
                               trace=trace)
    big = np.concatenate(
        [np.asarray(res.results[c]["out_t"]) for c in range(CORES)], axis=1)
    out = np.ascontiguousarray(big[:, gpos].T.astype(np.float32))
    return out, res


def kernel(**inputs):
    out, _ = _run(inputs, trace=False)
    return out
